# revision 28
# baseline (speedup 1.0000x reference)
"""Trainium2 Bass kernel for GQA attention with RoPE and block-diagonal
(document) causal masking, sharded over 8 NeuronCores by KV head group.

Per core c (of 8): Q heads 4c..4c+3, KV head c, both batches.
All matmuls in float32r (tf32-grade operand rounding, full PE rate at
moving-dim >= 256, fp32 PSUM accumulation).

v2 layout highlights:
  - x^T half-sequence (8 MB) resident in SBUF; projections run as 6
    sequential 16-matmul accumulation groups per half on a 3-bank PSUM
    rotation, with RoPE (DVE) trailing each group.
  - attention processes heads in PAIRS: one [128,512] scores matmul per
    key tile (multi-dim free AP over the two heads' Q), one exp, one
    [65,512] PV accumulation with an appended ones column for the
    softmax denominators; masking is a 0/1 multiply on partial tiles.
  - o_proj writes through a deep sbuf ring.
Host sums the 8 per-core partials.
"""
import sys
sys.path.insert(0, "/opt/trn_rl_repo")
import numpy as np

B, S, DIM = 2, 2048, 2048
NH, NKV, HD = 32, 8, 64
HPC = NH // 8           # 4 q-heads per core
MLOC = HPC * HD         # 256 local q dims
TQ = 256                # attention query chunk
NCORES = 8
NKC = DIM // 128        # 16 contraction chunks
NTC = S // 128          # 16 token chunks of 128
SCALE = 1.0 / 8.0

_nc_cache = {}


def _schedule(doc_ids):
    """Per batch: for each query chunk, the key-tile band and mask info."""
    doc = np.asarray(doc_ids)
    sched = []
    masks = []
    for b in range(B):
        d = doc[b]
        change = np.empty(S, dtype=np.int64)
        change[0] = 0
        idx = np.arange(1, S)
        change[1:] = np.where(d[1:] != d[:-1], idx, 0)
        start_idx = np.maximum.accumulate(change)
        per_qc = []
        for qc in range(S // TQ):
            q0 = qc * TQ
            t0 = int(start_idx[q0]) // 128
            t1 = (q0 + TQ) // 128
            row = []
            for kt in range(t0, t1):
                k0 = kt * 128
                full = (k0 + 127 <= q0 and d[k0] == d[k0 + 127] == d[q0] == d[q0 + TQ - 1])
                if full:
                    row.append((kt, None))
                else:
                    ks = np.arange(k0, k0 + 128)
                    qs = np.arange(q0, q0 + TQ)
                    m = (d[ks][:, None] == d[qs][None, :]) & (ks[:, None] <= qs[None, :])
                    masks.append(m.astype(np.float32))  # stacked later; cast in _prep
                    row.append((kt, len(masks) - 1))
            per_qc.append(row)
        sched.append(per_qc)
    if not masks:
        masks.append(np.zeros((128, TQ), np.float32))
    import ml_dtypes
    return sched, np.stack(masks).astype(ml_dtypes.bfloat16)


def _build_nc(sched, nmask):
    import concourse.bacc as bacc
    import concourse.mybir as mybir
    import concourse.tile as tile
    from concourse.masks import make_identity

    F32, F32R = mybir.dt.float32, mybir.dt.float32r
    Exp = mybir.ActivationFunctionType.Exp

    nc = bacc.Bacc()
    xT = nc.dram_tensor("xT", (B, DIM, S), F32, kind="ExternalInput")
    wq = nc.dram_tensor("wq", (DIM, MLOC), F32, kind="ExternalInput")
    wkv = nc.dram_tensor("wkv", (DIM, 128), F32, kind="ExternalInput")
    wo = nc.dram_tensor("wo", (MLOC, DIM), F32, kind="ExternalInput")
    cos128 = nc.dram_tensor("cos128", (128, S), F32, kind="ExternalInput")
    sin128 = nc.dram_tensor("sin128", (128, S), F32, kind="ExternalInput")
    masks = nc.dram_tensor("masks", (nmask, 128, TQ), mybir.dt.bfloat16, kind="ExternalInput")
    y = nc.dram_tensor("y", (B, S, DIM), F32, kind="ExternalOutput")

    with tile.TileContext(nc) as tc:
        with (
            tc.tile_pool(name="const", bufs=1) as cst,
            tc.tile_pool(name="xt", bufs=1) as xtp,
            tc.tile_pool(name="big", bufs=1) as big,
            tc.tile_pool(name="rope", bufs=3) as rp,
            tc.tile_pool(name="pt", bufs=4) as ptp,
            tc.tile_pool(name="mask", bufs=2) as mp,
            tc.tile_pool(name="small", bufs=3) as sp,
            tc.tile_pool(name="ysb", bufs=4) as yp,
            tc.tile_pool(name="pp", bufs=3, space="PSUM") as pp,
            tc.tile_pool(name="pa", bufs=5, space="PSUM") as pa,
        ):
            # ---- constants ----
            wq_sb = cst.tile([128, NKC, MLOC], F32R)
            for kc in range(NKC):
                nc.sync.dma_start(
                    wq_sb[:, kc, :], wq[kc * 128:(kc + 1) * 128, :].bitcast(F32R))
            wkv_sb = cst.tile([128, NKC, 128], F32R)
            for kc in range(NKC):
                nc.sync.dma_start(
                    wkv_sb[:, kc, :], wkv[kc * 128:(kc + 1) * 128, :].bitcast(F32R))
            wo_sb = cst.tile([128, 2, DIM], F32R)
            nc.sync.dma_start(wo_sb[:], wo[:].rearrange("(c p) m -> p c m", p=128).bitcast(F32R))
            cos_sb = cst.tile([128, S], F32)
            nc.sync.dma_start(cos_sb[:], cos128[:])
            sin_sb = cst.tile([128, S], F32)
            nc.sync.dma_start(sin_sb[:], sin128[:])
            ident = cst.tile([64, 64], F32)
            make_identity(nc, ident[:])
            scratch1 = cst.tile([128, 1], F32)
            nc.gpsimd.memset(scratch1[:], 1.0)
            ones64 = cst.tile([128, 64], F32R)
            nc.vector.tensor_copy(ones64[:], scratch1[:].broadcast_to([128, 64]))

            for b in range(B):
                # per-head-pair Q^T: [64, 2*S], head 2p+j at cols j*S..
                qrt = [big.tile([64, 2 * S], F32R, tag=f"qrt{m}", name=f"qrt{m}")
                       for m in range(2)]
                krt = big.tile([64, S], F32R, tag="krt")
                vaug = big.tile([128, NTC, 128], F32R, tag="vaug")
                or2t = [big.tile([128, S], F32R, tag=f"or2t{m}", name=f"or2t{m}")
                        for m in range(2)]

                # ================= projections + rope =================
                for tqi in range(4):
                    t0 = tqi * 512
                    tsl = slice(t0, t0 + 512)
                    xt = xtp.tile([128, NKC, 512], F32R, tag="xt")
                    for kc in range(NKC):
                        nc.sync.dma_start(
                            xt[:, kc, :],
                            xT[b, kc * 128:(kc + 1) * 128, t0:t0 + 512].bitcast(F32R))
                    for what in ("q0", "q1", "kv"):
                        gps = pp.tile([128, 512], F32, tag="pp", name=f"gps{b}{tqi}{what}")
                        for kc in range(NKC):
                            if what == "q0":
                                lhs = wq_sb[:, kc, 0:128]
                            elif what == "q1":
                                lhs = wq_sb[:, kc, 128:256]
                            else:
                                lhs = wkv_sb[:, kc, :]
                            nc.tensor.matmul(
                                gps[:], lhs, xt[:, kc, :],
                                start=(kc == 0), stop=(kc == NKC - 1))
                        if what in ("q0", "q1"):
                            m = 0 if what == "q0" else 1
                            gsb = rp.tile([128, 512], F32, tag="gsb")
                            nc.scalar.copy(gsb[:], gps[:])
                            tmp = rp.tile([128, 512], F32, tag="ra")
                            for blk in (0, 64):
                                nc.vector.tensor_mul(
                                    tmp[blk:blk + 32], gsb[blk + 32:blk + 64],
                                    sin_sb[blk + 32:blk + 64, tsl])
                                nc.vector.tensor_mul(
                                    tmp[blk + 32:blk + 64], gsb[blk:blk + 32],
                                    sin_sb[blk:blk + 32, tsl])
                            tmp2 = rp.tile([128, 512], F32, tag="rb")
                            nc.vector.tensor_mul(tmp2[:], gsb[:], cos_sb[:, tsl])
                            nc.vector.tensor_add(
                                qrt[m][:, t0:t0 + 512], tmp[0:64], tmp2[0:64])
                            nc.vector.tensor_add(
                                qrt[m][:, S + t0:S + t0 + 512], tmp[64:128], tmp2[64:128])
                        else:
                            gsb = rp.tile([128, 512], F32, tag="gsb")
                            nc.scalar.copy(gsb[:], gps[:])
                            tmp = rp.tile([64, 512], F32, tag="ra")
                            nc.vector.tensor_mul(tmp[0:32], gsb[32:64], sin_sb[32:64, tsl])
                            nc.vector.tensor_mul(tmp[32:64], gsb[0:32], sin_sb[0:32, tsl])
                            tmp2 = rp.tile([64, 512], F32, tag="rb")
                            nc.vector.tensor_mul(tmp2[:], gsb[0:64], cos_sb[0:64, tsl])
                            nc.vector.tensor_add(krt[:, tsl], tmp[:], tmp2[:])
                            vt = sp.tile([64, 512], F32, tag="vt")
                            nc.scalar.copy(vt[:], gsb[64:128])
                            for tc4 in range(4):
                                kt = (t0 // 128) + tc4
                                ptr = pa.tile([128, 64], F32, tag="pa", name=f"ptr{b}{tqi}{tc4}")
                                nc.tensor.transpose(
                                    ptr[:], vt[:, tc4 * 128:(tc4 + 1) * 128], ident[:])
                                nc.scalar.copy(vaug[:, kt, 0:64], ptr[:])
                                nc.scalar.copy(vaug[:, kt, 64:128], ones64[:])

                # ================= attention (head pairs) =================
                for qc in range(S // TQ):
                    q0 = qc * TQ
                    row = sched[b][qc]
                    mtiles = {}
                    for i, (kt, mi) in enumerate(row):
                        if mi is not None:
                            mt = mp.tile([128, TQ], mybir.dt.bfloat16, tag=f"m{i}", name=f"mt{i}")
                            nc.sync.dma_start(mt[:], masks[mi])
                            mtiles[kt] = mt
                    for pr in range(2):
                        # both heads' Q columns: [64, 2, TQ] multi-dim free AP
                        qpair = qrt[pr][:].rearrange("p (j s) -> p j s", j=2)[:, :, q0:q0 + TQ]
                        o_ps = pa.tile([128, 2 * TQ], F32, tag="pa", name=f"ops{b}{qc}{pr}")
                        for i, (kt, mi) in enumerate(row):
                            s_ps = pa.tile([128, 2 * TQ], F32, tag="pa", name=f"sps{b}{qc}{pr}{i}")
                            nc.tensor.matmul(
                                s_ps[:], krt[:, kt * 128:(kt + 1) * 128], qpair,
                                start=True, stop=True)
                            pt = ptp.tile([128, 2 * TQ], F32R, tag="pt")
                            nc.scalar.activation(pt[:], s_ps[:], Exp, scale=SCALE)
                            if mi is not None:
                                nc.vector.tensor_mul(pt[:, 0:TQ], pt[:, 0:TQ], mtiles[kt][:])
                                nc.vector.tensor_mul(pt[:, TQ:2 * TQ], pt[:, TQ:2 * TQ], mtiles[kt][:])
                            nc.tensor.matmul(
                                o_ps[:], vaug[:, kt, :], pt[:],
                                start=(i == 0), stop=(i == len(row) - 1))
                        zb = sp.tile([64, 2 * TQ], F32, tag="zb")
                        with nc.allow_low_precision(reason="normalization reciprocal"):
                            nc.vector.reciprocal(zb[:], o_ps[64:128, :])
                        qsl = slice(q0, q0 + TQ)
                        nc.vector.tensor_mul(
                            or2t[pr][0:64, qsl], o_ps[0:64, 0:TQ], zb[:, 0:TQ])
                        nc.vector.tensor_mul(
                            or2t[pr][64:128, qsl], o_ps[0:64, TQ:2 * TQ], zb[:, TQ:2 * TQ])
                    # o_proj for this query chunk's token tiles
                    for tc_ in range(qc * (TQ // 128), (qc + 1) * (TQ // 128)):
                        for mc in range(4):
                            y_ps = pa.tile([128, 512], F32, tag="pa", name=f"yps{b}{tc_}{mc}")
                            for hp in range(2):
                                nc.tensor.matmul(
                                    y_ps[:], or2t[hp][:, tc_ * 128:(tc_ + 1) * 128],
                                    wo_sb[:, hp, mc * 512:(mc + 1) * 512],
                                    start=(hp == 0), stop=(hp == 1))
                            y_sb = yp.tile([128, 512], F32, tag="ysb")
                            nc.scalar.copy(y_sb[:], y_ps[:])
                            nc.sync.dma_start(
                                y[b, tc_ * 128:(tc_ + 1) * 128, mc * 512:(mc + 1) * 512],
                                y_sb[:])


    nc.finalize()
    return nc


def _prep_inputs(x, rope_cos, rope_sin, doc_ids, Wq, Wk, Wv, Wo):
    x = np.asarray(x, np.float32)
    xT = np.ascontiguousarray(x.transpose(0, 2, 1))
    cosT = np.asarray(rope_cos, np.float32).T          # (32, S)
    sinT = np.asarray(rope_sin, np.float32).T
    cos128 = np.tile(np.concatenate([cosT, cosT], 0), (2, 1))      # (128, S)
    sin128 = np.tile(np.concatenate([sinT, -sinT], 0), (2, 1))
    sched, masks = _schedule(doc_ids)
    Wq = np.asarray(Wq, np.float32)
    Wk = np.asarray(Wk, np.float32)
    Wv = np.asarray(Wv, np.float32)
    Wo = np.asarray(Wo, np.float32)
    in_maps = []
    for c in range(NCORES):
        wq_c = np.ascontiguousarray(Wq[c * MLOC:(c + 1) * MLOC].T)      # (DIM, 256)
        wk_c = Wk[c * HD:(c + 1) * HD].T                                # (DIM, 64)
        wv_c = Wv[c * HD:(c + 1) * HD].T
        wkv_c = np.ascontiguousarray(np.concatenate([wk_c, wv_c], 1))   # (DIM, 128)
        wo_c = np.ascontiguousarray(Wo[:, c * MLOC:(c + 1) * MLOC].T)   # (256, DIM)
        in_maps.append({
            "xT": xT, "wq": wq_c, "wkv": wkv_c, "wo": wo_c,
            "cos128": cos128, "sin128": sin128, "masks": masks,
        })
    return sched, masks, in_maps


def kernel(x, rope_cos, rope_sin, doc_ids, Wq, Wk, Wv, Wo):
    from concourse.bass_utils import run_bass_kernel_spmd
    sched, masks, in_maps = _prep_inputs(
        x, rope_cos, rope_sin, doc_ids, Wq, Wk, Wv, Wo)
    key = (tuple(tuple(tuple((kt, mi is not None) for kt, mi in row) for row in sb)
                 for sb in sched), masks.shape[0])
    nc = _nc_cache.get(key)
    if nc is None:
        nc = _build_nc(sched, masks.shape[0])
        _nc_cache[key] = nc
    res = run_bass_kernel_spmd(nc, in_maps, core_ids=list(range(NCORES)))
    y = np.zeros((B, S, DIM), np.float32)
    for c in range(NCORES):
        y += res.results[c]["y"]
    return y


# revision 29
# speedup vs baseline: 78.3076x; 78.3076x over previous
"""Trainium2 Bass kernel for GQA attention with RoPE and block-diagonal
(document) causal masking, sharded over 8 NeuronCores by KV head group.

Per core c (of 8): Q heads 4c..4c+3, KV head c, both batches.
All matmuls in float32r (tf32-grade operand rounding, full PE rate at
moving-dim >= 256, fp32 PSUM accumulation).

v2 layout highlights:
  - x^T half-sequence (8 MB) resident in SBUF; projections run as 6
    sequential 16-matmul accumulation groups per half on a 3-bank PSUM
    rotation, with RoPE (DVE) trailing each group.
  - attention processes heads in PAIRS: one [128,512] scores matmul per
    key tile (multi-dim free AP over the two heads' Q), one exp, one
    [65,512] PV accumulation with an appended ones column for the
    softmax denominators; masking is a 0/1 multiply on partial tiles.
  - o_proj writes through a deep sbuf ring.
Host sums the 8 per-core partials.
"""
import sys
sys.path.insert(0, "/opt/trn_rl_repo")
import numpy as np

B, S, DIM = 2, 2048, 2048
NH, NKV, HD = 32, 8, 64
HPC = NH // 8           # 4 q-heads per core
MLOC = HPC * HD         # 256 local q dims
TQ = 256                # attention query chunk
NCORES = 8
NKC = DIM // 128        # 16 contraction chunks
NTC = S // 128          # 16 token chunks of 128
SCALE = 1.0 / 8.0

_nc_cache = {}


def _schedule(doc_ids):
    """Per batch: for each query chunk, the key-tile band and mask info."""
    doc = np.asarray(doc_ids)
    sched = []
    masks = []
    for b in range(B):
        d = doc[b]
        change = np.empty(S, dtype=np.int64)
        change[0] = 0
        idx = np.arange(1, S)
        change[1:] = np.where(d[1:] != d[:-1], idx, 0)
        start_idx = np.maximum.accumulate(change)
        per_qc = []
        for qc in range(S // TQ):
            q0 = qc * TQ
            t0 = int(start_idx[q0]) // 128
            t1 = (q0 + TQ) // 128
            row = []
            for kt in range(t0, t1):
                k0 = kt * 128
                full = (k0 + 127 <= q0 and d[k0] == d[k0 + 127] == d[q0] == d[q0 + TQ - 1])
                if full:
                    row.append((kt, None))
                else:
                    ks = np.arange(k0, k0 + 128)
                    qs = np.arange(q0, q0 + TQ)
                    m = (d[ks][:, None] == d[qs][None, :]) & (ks[:, None] <= qs[None, :])
                    masks.append(m.astype(np.float32))  # stacked later; cast in _prep
                    row.append((kt, len(masks) - 1))
            per_qc.append(row)
        sched.append(per_qc)
    if not masks:
        masks.append(np.zeros((128, TQ), np.float32))
    import ml_dtypes
    return sched, np.stack(masks).astype(ml_dtypes.bfloat16)


def _build_nc(sched, nmask):
    import concourse.bacc as bacc
    import concourse.mybir as mybir
    import concourse.tile as tile
    from concourse.masks import make_identity

    F32, F32R = mybir.dt.float32, mybir.dt.float32r
    Exp = mybir.ActivationFunctionType.Exp

    nc = bacc.Bacc()
    xT = nc.dram_tensor("xT", (B, DIM, S), F32, kind="ExternalInput")
    wq = nc.dram_tensor("wq", (DIM, MLOC), F32, kind="ExternalInput")
    wkv = nc.dram_tensor("wkv", (DIM, 128), F32, kind="ExternalInput")
    wo = nc.dram_tensor("wo", (MLOC, DIM), F32, kind="ExternalInput")
    cos128 = nc.dram_tensor("cos128", (128, S), F32, kind="ExternalInput")
    sin128 = nc.dram_tensor("sin128", (128, S), F32, kind="ExternalInput")
    masks = nc.dram_tensor("masks", (nmask, 128, TQ), mybir.dt.bfloat16, kind="ExternalInput")
    y = nc.dram_tensor("y", (B, S, DIM), F32, kind="ExternalOutput")

    with tile.TileContext(nc) as tc:
        with (
            tc.tile_pool(name="const", bufs=1) as cst,
            tc.tile_pool(name="xt", bufs=1) as xtp,
            tc.tile_pool(name="big", bufs=1) as big,
            tc.tile_pool(name="rope", bufs=3) as rp,
            tc.tile_pool(name="pt", bufs=4) as ptp,
            tc.tile_pool(name="mask", bufs=2) as mp,
            tc.tile_pool(name="small", bufs=3) as sp,
            tc.tile_pool(name="ysb", bufs=4) as yp,
            tc.tile_pool(name="pp", bufs=3, space="PSUM") as pp,
            tc.tile_pool(name="pa", bufs=5, space="PSUM") as pa,
        ):
            # ---- constants ----
            wq_sb = cst.tile([128, NKC, MLOC], F32R)
            for kc in range(NKC):
                nc.sync.dma_start(
                    wq_sb[:, kc, :], wq[kc * 128:(kc + 1) * 128, :].bitcast(F32R))
            wkv_sb = cst.tile([128, NKC, 128], F32R)
            for kc in range(NKC):
                nc.sync.dma_start(
                    wkv_sb[:, kc, :], wkv[kc * 128:(kc + 1) * 128, :].bitcast(F32R))
            wo_sb = cst.tile([128, 2, DIM], F32R)
            nc.sync.dma_start(wo_sb[:], wo[:].rearrange("(c p) m -> p c m", p=128).bitcast(F32R))
            cos_sb = cst.tile([128, S], F32)
            nc.sync.dma_start(cos_sb[:], cos128[:])
            sin_sb = cst.tile([128, S], F32)
            nc.sync.dma_start(sin_sb[:], sin128[:])
            ident = cst.tile([64, 64], F32)
            make_identity(nc, ident[:])
            scratch1 = cst.tile([128, 1], F32)
            nc.gpsimd.memset(scratch1[:], 1.0)
            ones64 = cst.tile([128, 64], F32R)
            nc.vector.tensor_copy(ones64[:], scratch1[:].broadcast_to([128, 64]))

            for b in range(B):
                # per-head-pair Q^T: [64, 2*S], head 2p+j at cols j*S..
                qrt = [big.tile([64, 2 * S], F32R, tag=f"qrt{m}", name=f"qrt{m}")
                       for m in range(2)]
                krt = big.tile([64, S], F32R, tag="krt")
                vaug = big.tile([128, NTC, 128], F32R, tag="vaug")
                or2t = [big.tile([128, S], F32R, tag=f"or2t{m}", name=f"or2t{m}")
                        for m in range(2)]

                # ================= projections + rope =================
                for tqi in range(4):
                    t0 = tqi * 512
                    tsl = slice(t0, t0 + 512)
                    xt = xtp.tile([128, NKC, 512], F32R, tag="xt")
                    for kc in range(NKC):
                        nc.sync.dma_start(
                            xt[:, kc, :],
                            xT[b, kc * 128:(kc + 1) * 128, t0:t0 + 512].bitcast(F32R))
                    for what in ("q0", "q1", "kv"):
                        gps = pp.tile([128, 512], F32, tag="pp", name=f"gps{b}{tqi}{what}")
                        for kc in range(NKC):
                            if what == "q0":
                                lhs = wq_sb[:, kc, 0:128]
                            elif what == "q1":
                                lhs = wq_sb[:, kc, 128:256]
                            else:
                                lhs = wkv_sb[:, kc, :]
                            nc.tensor.matmul(
                                gps[:], lhs, xt[:, kc, :],
                                start=(kc == 0), stop=(kc == NKC - 1))
                        if what in ("q0", "q1"):
                            m = 0 if what == "q0" else 1
                            gsb = rp.tile([128, 512], F32, tag="gsb")
                            nc.scalar.copy(gsb[:], gps[:])
                            tmp = rp.tile([128, 512], F32, tag="ra")
                            for blk in (0, 64):
                                nc.vector.tensor_mul(
                                    tmp[blk:blk + 32], gsb[blk + 32:blk + 64],
                                    sin_sb[blk + 32:blk + 64, tsl])
                                nc.vector.tensor_mul(
                                    tmp[blk + 32:blk + 64], gsb[blk:blk + 32],
                                    sin_sb[blk:blk + 32, tsl])
                            tmp2 = rp.tile([128, 512], F32, tag="rb")
                            nc.vector.tensor_mul(tmp2[:], gsb[:], cos_sb[:, tsl])
                            nc.vector.tensor_add(
                                qrt[m][:, t0:t0 + 512], tmp[0:64], tmp2[0:64])
                            nc.vector.tensor_add(
                                qrt[m][:, S + t0:S + t0 + 512], tmp[64:128], tmp2[64:128])
                        else:
                            gsb = rp.tile([128, 512], F32, tag="gsb")
                            nc.scalar.copy(gsb[:], gps[:])
                            tmp = rp.tile([64, 512], F32, tag="ra")
                            nc.vector.tensor_mul(tmp[0:32], gsb[32:64], sin_sb[32:64, tsl])
                            nc.vector.tensor_mul(tmp[32:64], gsb[0:32], sin_sb[0:32, tsl])
                            tmp2 = rp.tile([64, 512], F32, tag="rb")
                            nc.vector.tensor_mul(tmp2[:], gsb[0:64], cos_sb[0:64, tsl])
                            nc.vector.tensor_add(krt[:, tsl], tmp[:], tmp2[:])
                            vt = sp.tile([64, 512], F32, tag="vt")
                            nc.scalar.copy(vt[:], gsb[64:128])
                            for tc4 in range(4):
                                kt = (t0 // 128) + tc4
                                ptr = pa.tile([128, 64], F32, tag="pa", name=f"ptr{b}{tqi}{tc4}")
                                nc.tensor.transpose(
                                    ptr[:], vt[:, tc4 * 128:(tc4 + 1) * 128], ident[:])
                                nc.scalar.copy(vaug[:, kt, 0:64], ptr[:])
                                nc.scalar.copy(vaug[:, kt, 64:128], ones64[:])

                # ================= attention (head pairs) =================
                for qc in range(S // TQ):
                    q0 = qc * TQ
                    row = sched[b][qc]
                    mtiles = {}
                    for i, (kt, mi) in enumerate(row):
                        if mi is not None:
                            mt = mp.tile([128, TQ], mybir.dt.bfloat16, tag=f"m{i % 6}", name=f"mt{i}")
                            nc.sync.dma_start(mt[:], masks[mi])
                            mtiles[kt] = mt
                    for pr in range(2):
                        # both heads' Q columns: [64, 2, TQ] multi-dim free AP
                        qpair = qrt[pr][:].rearrange("p (j s) -> p j s", j=2)[:, :, q0:q0 + TQ]
                        o_ps = pa.tile([128, 2 * TQ], F32, tag="pa", name=f"ops{b}{qc}{pr}")
                        for i, (kt, mi) in enumerate(row):
                            s_ps = pa.tile([128, 2 * TQ], F32, tag="pa", name=f"sps{b}{qc}{pr}{i}")
                            nc.tensor.matmul(
                                s_ps[:], krt[:, kt * 128:(kt + 1) * 128], qpair,
                                start=True, stop=True)
                            pt = ptp.tile([128, 2 * TQ], F32R, tag="pt")
                            nc.scalar.activation(pt[:], s_ps[:], Exp, scale=SCALE)
                            if mi is not None:
                                nc.vector.tensor_mul(pt[:, 0:TQ], pt[:, 0:TQ], mtiles[kt][:])
                                nc.vector.tensor_mul(pt[:, TQ:2 * TQ], pt[:, TQ:2 * TQ], mtiles[kt][:])
                            nc.tensor.matmul(
                                o_ps[:], vaug[:, kt, :], pt[:],
                                start=(i == 0), stop=(i == len(row) - 1))
                        zb = sp.tile([64, 2 * TQ], F32, tag="zb")
                        with nc.allow_low_precision(reason="normalization reciprocal"):
                            nc.vector.reciprocal(zb[:], o_ps[64:128, :])
                        qsl = slice(q0, q0 + TQ)
                        nc.vector.tensor_mul(
                            or2t[pr][0:64, qsl], o_ps[0:64, 0:TQ], zb[:, 0:TQ])
                        nc.vector.tensor_mul(
                            or2t[pr][64:128, qsl], o_ps[0:64, TQ:2 * TQ], zb[:, TQ:2 * TQ])
                    # o_proj for this query chunk's token tiles
                    for tc_ in range(qc * (TQ // 128), (qc + 1) * (TQ // 128)):
                        for mc in range(4):
                            y_ps = pa.tile([128, 512], F32, tag="pa", name=f"yps{b}{tc_}{mc}")
                            for hp in range(2):
                                nc.tensor.matmul(
                                    y_ps[:], or2t[hp][:, tc_ * 128:(tc_ + 1) * 128],
                                    wo_sb[:, hp, mc * 512:(mc + 1) * 512],
                                    start=(hp == 0), stop=(hp == 1))
                            y_sb = yp.tile([128, 512], F32, tag="ysb")
                            nc.scalar.copy(y_sb[:], y_ps[:])
                            nc.sync.dma_start(
                                y[b, tc_ * 128:(tc_ + 1) * 128, mc * 512:(mc + 1) * 512],
                                y_sb[:])


    nc.finalize()
    return nc


def _prep_inputs(x, rope_cos, rope_sin, doc_ids, Wq, Wk, Wv, Wo):
    x = np.asarray(x, np.float32)
    xT = np.ascontiguousarray(x.transpose(0, 2, 1))
    cosT = np.asarray(rope_cos, np.float32).T          # (32, S)
    sinT = np.asarray(rope_sin, np.float32).T
    cos128 = np.tile(np.concatenate([cosT, cosT], 0), (2, 1))      # (128, S)
    sin128 = np.tile(np.concatenate([sinT, -sinT], 0), (2, 1))
    sched, masks = _schedule(doc_ids)
    Wq = np.asarray(Wq, np.float32)
    Wk = np.asarray(Wk, np.float32)
    Wv = np.asarray(Wv, np.float32)
    Wo = np.asarray(Wo, np.float32)
    in_maps = []
    for c in range(NCORES):
        wq_c = np.ascontiguousarray(Wq[c * MLOC:(c + 1) * MLOC].T)      # (DIM, 256)
        wk_c = Wk[c * HD:(c + 1) * HD].T                                # (DIM, 64)
        wv_c = Wv[c * HD:(c + 1) * HD].T
        wkv_c = np.ascontiguousarray(np.concatenate([wk_c, wv_c], 1))   # (DIM, 128)
        wo_c = np.ascontiguousarray(Wo[:, c * MLOC:(c + 1) * MLOC].T)   # (256, DIM)
        in_maps.append({
            "xT": xT, "wq": wq_c, "wkv": wkv_c, "wo": wo_c,
            "cos128": cos128, "sin128": sin128, "masks": masks,
        })
    return sched, masks, in_maps


def kernel(x, rope_cos, rope_sin, doc_ids, Wq, Wk, Wv, Wo):
    from concourse.bass_utils import run_bass_kernel_spmd
    sched, masks, in_maps = _prep_inputs(
        x, rope_cos, rope_sin, doc_ids, Wq, Wk, Wv, Wo)
    key = (tuple(tuple(tuple((kt, mi is not None) for kt, mi in row) for row in sb)
                 for sb in sched), masks.shape[0])
    nc = _nc_cache.get(key)
    if nc is None:
        nc = _build_nc(sched, masks.shape[0])
        _nc_cache[key] = nc
    res = run_bass_kernel_spmd(nc, in_maps, core_ids=list(range(NCORES)))
    y = np.zeros((B, S, DIM), np.float32)
    for c in range(NCORES):
        y += res.results[c]["y"]
    return y


# revision 32
# speedup vs baseline: 140.8631x; 1.7988x over previous
"""Trainium2 Bass kernel for GQA attention with RoPE and block-diagonal
(document) causal masking, sharded over 8 NeuronCores by KV head group.

Per core c (of 8): Q heads 4c..4c+3, KV head c, both batches.
All matmuls in float32r (tf32-grade operand rounding, full PE rate at
moving-dim >= 256, fp32 PSUM accumulation).

v2 layout highlights:
  - x^T half-sequence (8 MB) resident in SBUF; projections run as 6
    sequential 16-matmul accumulation groups per half on a 3-bank PSUM
    rotation, with RoPE (DVE) trailing each group.
  - attention processes heads in PAIRS: one [128,512] scores matmul per
    key tile (multi-dim free AP over the two heads' Q), one exp, one
    [65,512] PV accumulation with an appended ones column for the
    softmax denominators; masking is a 0/1 multiply on partial tiles.
  - o_proj writes through a deep sbuf ring.
Host sums the 8 per-core partials.
"""
import sys
sys.path.insert(0, "/opt/trn_rl_repo")
import numpy as np

B, S, DIM = 2, 2048, 2048
NH, NKV, HD = 32, 8, 64
HPC = NH // 8           # 4 q-heads per core
MLOC = HPC * HD         # 256 local q dims
TQ = 256                # attention query chunk
NCORES = 8
NKC = DIM // 128        # 16 contraction chunks
NTC = S // 128          # 16 token chunks of 128
SCALE = 1.0 / 8.0

_nc_cache = {}


def _schedule(doc_ids):
    """Per batch: for each query chunk, the key-tile band and mask info."""
    doc = np.asarray(doc_ids)
    sched = []
    masks = []
    for b in range(B):
        d = doc[b]
        change = np.empty(S, dtype=np.int64)
        change[0] = 0
        idx = np.arange(1, S)
        change[1:] = np.where(d[1:] != d[:-1], idx, 0)
        start_idx = np.maximum.accumulate(change)
        per_qc = []
        for qc in range(S // TQ):
            q0 = qc * TQ
            t0 = int(start_idx[q0]) // 128
            t1 = (q0 + TQ) // 128
            row = []
            for kt in range(t0, t1):
                k0 = kt * 128
                full = (k0 + 127 <= q0 and d[k0] == d[k0 + 127] == d[q0] == d[q0 + TQ - 1])
                if full:
                    row.append((kt, None))
                else:
                    ks = np.arange(k0, k0 + 128)
                    qs = np.arange(q0, q0 + TQ)
                    m = (d[ks][:, None] == d[qs][None, :]) & (ks[:, None] <= qs[None, :])
                    masks.append(m.astype(np.float32))  # stacked later; cast in _prep
                    row.append((kt, len(masks) - 1))
            per_qc.append(row)
        sched.append(per_qc)
    if not masks:
        masks.append(np.zeros((128, TQ), np.float32))
    import ml_dtypes
    return sched, np.stack(masks).astype(ml_dtypes.bfloat16)


def _build_nc(sched, nmask):
    import concourse.bacc as bacc
    import concourse.mybir as mybir
    import concourse.tile as tile
    from concourse.masks import make_identity

    F32, F32R = mybir.dt.float32, mybir.dt.float32r
    Exp = mybir.ActivationFunctionType.Exp

    nc = bacc.Bacc()
    xT = nc.dram_tensor("xT", (B, DIM, S), F32, kind="ExternalInput")
    wq = nc.dram_tensor("wq", (DIM, MLOC), F32, kind="ExternalInput")
    wkv = nc.dram_tensor("wkv", (DIM, 128), F32, kind="ExternalInput")
    wo = nc.dram_tensor("wo", (MLOC, DIM), F32, kind="ExternalInput")
    cos128 = nc.dram_tensor("cos128", (128, S), F32, kind="ExternalInput")
    sin128 = nc.dram_tensor("sin128", (128, S), F32, kind="ExternalInput")
    masks = nc.dram_tensor("masks", (nmask, 128, TQ), mybir.dt.bfloat16, kind="ExternalInput")
    y = nc.dram_tensor("y", (B, S, DIM), F32, kind="ExternalOutput")

    with tile.TileContext(nc) as tc:
        with (
            tc.tile_pool(name="const", bufs=1) as cst,
            tc.tile_pool(name="xt", bufs=1) as xtp,
            tc.tile_pool(name="big", bufs=1) as big,
            tc.tile_pool(name="rope", bufs=3) as rp,
            tc.tile_pool(name="pt", bufs=4) as ptp,
            tc.tile_pool(name="mask", bufs=2) as mp,
            tc.tile_pool(name="small", bufs=3) as sp,
            tc.tile_pool(name="ysb", bufs=4) as yp,
            tc.tile_pool(name="pp", bufs=3, space="PSUM") as pp,
            tc.tile_pool(name="pa", bufs=5, space="PSUM") as pa,
        ):
            # ---- constants ----
            wq_sb = cst.tile([128, NKC, MLOC], F32R)
            for kc in range(NKC):
                nc.sync.dma_start(
                    wq_sb[:, kc, :], wq[kc * 128:(kc + 1) * 128, :].bitcast(F32R))
            wkv_sb = cst.tile([128, NKC, 128], F32R)
            for kc in range(NKC):
                nc.sync.dma_start(
                    wkv_sb[:, kc, :], wkv[kc * 128:(kc + 1) * 128, :].bitcast(F32R))
            wo_sb = cst.tile([128, 2, DIM], F32R)
            nc.sync.dma_start(wo_sb[:], wo[:].rearrange("(c p) m -> p c m", p=128).bitcast(F32R))
            cos_sb = cst.tile([128, S], F32)
            nc.sync.dma_start(cos_sb[:], cos128[:])
            sin_sb = cst.tile([128, S], F32)
            nc.sync.dma_start(sin_sb[:], sin128[:])
            ident = cst.tile([64, 64], F32)
            make_identity(nc, ident[:])
            scratch1 = cst.tile([128, 1], F32)
            nc.gpsimd.memset(scratch1[:], 1.0)
            ones64 = cst.tile([128, 64], F32R)
            nc.vector.tensor_copy(ones64[:], scratch1[:].broadcast_to([128, 64]))

            for b in range(B):
                # per-head-pair Q^T: [64, 2*S], head 2p+j at cols j*S..
                qrt = [big.tile([64, 2 * S], F32R, tag=f"qrt{m}", name=f"qrt{m}")
                       for m in range(2)]
                krt = big.tile([64, S], F32R, tag="krt")
                vaug = big.tile([128, NTC, 128], F32R, tag="vaug")
                or2t = [big.tile([128, S], F32R, tag=f"or2t{m}", name=f"or2t{m}")
                        for m in range(2)]

                # ================= projections + rope =================
                for tqi in range(4):
                    t0 = tqi * 512
                    tsl = slice(t0, t0 + 512)
                    xt_lo = xtp.tile([128, NKC // 2, 512], F32R, tag="xtlo", name=f"xtlo{b}{tqi}")
                    xt_hi = xtp.tile([128, NKC // 2, 512], F32R, tag="xthi", name=f"xthi{b}{tqi}")
                    for kc in range(NKC):
                        dst = xt_lo if kc < NKC // 2 else xt_hi
                        nc.sync.dma_start(
                            dst[:, kc % (NKC // 2), :],
                            xT[b, kc * 128:(kc + 1) * 128, t0:t0 + 512].bitcast(F32R))
                    for what in ("q0", "q1", "kv"):
                        gps = pp.tile([128, 512], F32, tag="pp", name=f"gps{b}{tqi}{what}")
                        for kc in range(NKC):
                            if what == "q0":
                                lhs = wq_sb[:, kc, 0:128]
                            elif what == "q1":
                                lhs = wq_sb[:, kc, 128:256]
                            else:
                                lhs = wkv_sb[:, kc, :]
                            xsrc = xt_lo if kc < NKC // 2 else xt_hi
                            nc.tensor.matmul(
                                gps[:], lhs, xsrc[:, kc % (NKC // 2), :],
                                start=(kc == 0), stop=(kc == NKC - 1))
                        if what in ("q0", "q1"):
                            m = 0 if what == "q0" else 1
                            gsb = rp.tile([128, 512], F32, tag="gsb")
                            nc.scalar.copy(gsb[:], gps[:])
                            tmp = rp.tile([128, 512], F32, tag="ra")
                            for blk in (0, 64):
                                nc.vector.tensor_mul(
                                    tmp[blk:blk + 32], gsb[blk + 32:blk + 64],
                                    sin_sb[blk + 32:blk + 64, tsl])
                                nc.vector.tensor_mul(
                                    tmp[blk + 32:blk + 64], gsb[blk:blk + 32],
                                    sin_sb[blk:blk + 32, tsl])
                            tmp2 = rp.tile([128, 512], F32, tag="rb")
                            nc.vector.tensor_mul(tmp2[:], gsb[:], cos_sb[:, tsl])
                            nc.vector.tensor_add(
                                qrt[m][:, t0:t0 + 512], tmp[0:64], tmp2[0:64])
                            nc.vector.tensor_add(
                                qrt[m][:, S + t0:S + t0 + 512], tmp[64:128], tmp2[64:128])
                        else:
                            gsb = rp.tile([128, 512], F32, tag="gsb")
                            nc.scalar.copy(gsb[:], gps[:])
                            tmp = rp.tile([64, 512], F32, tag="ra")
                            nc.vector.tensor_mul(tmp[0:32], gsb[32:64], sin_sb[32:64, tsl])
                            nc.vector.tensor_mul(tmp[32:64], gsb[0:32], sin_sb[0:32, tsl])
                            tmp2 = rp.tile([64, 512], F32, tag="rb")
                            nc.vector.tensor_mul(tmp2[:], gsb[0:64], cos_sb[0:64, tsl])
                            nc.vector.tensor_add(krt[:, tsl], tmp[:], tmp2[:])
                            vt = sp.tile([64, 512], F32, tag="vt")
                            nc.scalar.copy(vt[:], gsb[64:128])
                            for tc4 in range(4):
                                kt = (t0 // 128) + tc4
                                ptr = pa.tile([128, 64], F32, tag="pa", name=f"ptr{b}{tqi}{tc4}")
                                nc.tensor.transpose(
                                    ptr[:], vt[:, tc4 * 128:(tc4 + 1) * 128], ident[:])
                                nc.scalar.copy(vaug[:, kt, 0:64], ptr[:])
                                nc.scalar.copy(vaug[:, kt, 64:128], ones64[:])

                # ================= attention (head pairs) =================
                for qc in range(S // TQ):
                    q0 = qc * TQ
                    row = sched[b][qc]
                    mtiles = {}
                    for i, (kt, mi) in enumerate(row):
                        if mi is not None:
                            mt = mp.tile([128, TQ], mybir.dt.bfloat16, tag=f"m{i % 6}", name=f"mt{i}")
                            nc.sync.dma_start(mt[:], masks[mi])
                            mtiles[kt] = mt
                    for pr in range(2):
                        # both heads' Q columns: [64, 2, TQ] multi-dim free AP
                        qpair = qrt[pr][:].rearrange("p (j s) -> p j s", j=2)[:, :, q0:q0 + TQ]
                        o_ps = pa.tile([128, 2 * TQ], F32, tag="pa", name=f"ops{b}{qc}{pr}")
                        for i, (kt, mi) in enumerate(row):
                            s_ps = pa.tile([128, 2 * TQ], F32, tag="pa", name=f"sps{b}{qc}{pr}{i}")
                            nc.tensor.matmul(
                                s_ps[:], krt[:, kt * 128:(kt + 1) * 128], qpair,
                                start=True, stop=True)
                            pt = ptp.tile([128, 2 * TQ], F32R, tag="pt")
                            nc.scalar.activation(pt[:], s_ps[:], Exp, scale=SCALE)
                            if mi is not None:
                                nc.vector.tensor_mul(pt[:, 0:TQ], pt[:, 0:TQ], mtiles[kt][:])
                                nc.vector.tensor_mul(pt[:, TQ:2 * TQ], pt[:, TQ:2 * TQ], mtiles[kt][:])
                            nc.tensor.matmul(
                                o_ps[:], vaug[:, kt, :], pt[:],
                                start=(i == 0), stop=(i == len(row) - 1))
                        zb = sp.tile([64, 2 * TQ], F32, tag="zb")
                        with nc.allow_low_precision(reason="normalization reciprocal"):
                            nc.vector.reciprocal(zb[:], o_ps[64:128, :])
                        qsl = slice(q0, q0 + TQ)
                        nc.vector.tensor_mul(
                            or2t[pr][0:64, qsl], o_ps[0:64, 0:TQ], zb[:, 0:TQ])
                        nc.vector.tensor_mul(
                            or2t[pr][64:128, qsl], o_ps[0:64, TQ:2 * TQ], zb[:, TQ:2 * TQ])
                    # o_proj for this query chunk's token tiles
                    for tc_ in range(qc * (TQ // 128), (qc + 1) * (TQ // 128)):
                        for mc in range(4):
                            y_ps = pa.tile([128, 512], F32, tag="pa", name=f"yps{b}{tc_}{mc}")
                            for hp in range(2):
                                nc.tensor.matmul(
                                    y_ps[:], or2t[hp][:, tc_ * 128:(tc_ + 1) * 128],
                                    wo_sb[:, hp, mc * 512:(mc + 1) * 512],
                                    start=(hp == 0), stop=(hp == 1))
                            y_sb = yp.tile([128, 512], F32, tag="ysb")
                            nc.scalar.copy(y_sb[:], y_ps[:])
                            nc.sync.dma_start(
                                y[b, tc_ * 128:(tc_ + 1) * 128, mc * 512:(mc + 1) * 512],
                                y_sb[:])


    nc.finalize()
    return nc


def _prep_inputs(x, rope_cos, rope_sin, doc_ids, Wq, Wk, Wv, Wo):
    x = np.asarray(x, np.float32)
    xT = np.ascontiguousarray(x.transpose(0, 2, 1))
    cosT = np.asarray(rope_cos, np.float32).T          # (32, S)
    sinT = np.asarray(rope_sin, np.float32).T
    cos128 = np.tile(np.concatenate([cosT, cosT], 0), (2, 1))      # (128, S)
    sin128 = np.tile(np.concatenate([sinT, -sinT], 0), (2, 1))
    sched, masks = _schedule(doc_ids)
    Wq = np.asarray(Wq, np.float32)
    Wk = np.asarray(Wk, np.float32)
    Wv = np.asarray(Wv, np.float32)
    Wo = np.asarray(Wo, np.float32)
    in_maps = []
    for c in range(NCORES):
        wq_c = np.ascontiguousarray(Wq[c * MLOC:(c + 1) * MLOC].T)      # (DIM, 256)
        wk_c = Wk[c * HD:(c + 1) * HD].T                                # (DIM, 64)
        wv_c = Wv[c * HD:(c + 1) * HD].T
        wkv_c = np.ascontiguousarray(np.concatenate([wk_c, wv_c], 1))   # (DIM, 128)
        wo_c = np.ascontiguousarray(Wo[:, c * MLOC:(c + 1) * MLOC].T)   # (256, DIM)
        in_maps.append({
            "xT": xT, "wq": wq_c, "wkv": wkv_c, "wo": wo_c,
            "cos128": cos128, "sin128": sin128, "masks": masks,
        })
    return sched, masks, in_maps


def kernel(x, rope_cos, rope_sin, doc_ids, Wq, Wk, Wv, Wo):
    from concourse.bass_utils import run_bass_kernel_spmd
    sched, masks, in_maps = _prep_inputs(
        x, rope_cos, rope_sin, doc_ids, Wq, Wk, Wv, Wo)
    key = (tuple(tuple(tuple((kt, mi is not None) for kt, mi in row) for row in sb)
                 for sb in sched), masks.shape[0])
    nc = _nc_cache.get(key)
    if nc is None:
        nc = _build_nc(sched, masks.shape[0])
        _nc_cache[key] = nc
    res = run_bass_kernel_spmd(nc, in_maps, core_ids=list(range(NCORES)))
    y = np.zeros((B, S, DIM), np.float32)
    for c in range(NCORES):
        y += res.results[c]["y"]
    return y
